# revision 22
# baseline (speedup 1.0000x reference)
"""LSTM cell (4-gate) Trainium2 Bass kernel, data-parallel over batch on 8 cores.

Computation (per reference):
    ih = concat(i, h, axis=1)                 # [B, K], K = 4096
    o_g = act_g(ih @ Wg.T + bg)               # gates, act = sigmoid/sigmoid/tanh/sigmoid
    new_c = c*o1 + o2*o3
    new_h = tanh(c) * o4

Strategy: shard batch B=8192 across 8 cores (1024 rows each); weights replicated.
All matmuls run in the transposed domain: out[j, b] = sum_k W_g[j, k] * ihT[k, b],
with the weight tile stationary and ihT moving [128k x 512b].  The gate bias is a
per-partition vector fused into the ScalarE activation.

Mixed precision, tuned PER GATE to the 2e-2 error budget: the first K8S[g]
columns of gate g's contraction run as fp8-e4m3 DoubleRow matmuls (256-wide
contraction per instruction at 2x PE rate); the rest run in bf16.  Gate
sensitivities differ a lot: o2's error is doubly damped (sigmoid' and |o3|<1)
so gate 1 is pure fp8; o1 is amplified by c (tail ~5.4) and o3 by tanh'=1, so
gates 0/2 keep larger bf16 suffixes; o4 alone sets the h error.  ALL weights
are pre-scaled by 2^12 on the host so fp8 weights sit in e4m3's normal range
while both partial products accumulate at the same scale in one PSUM bank; the
ScalarE activation applies scale=2^-12 for free.  ih is quantized to e4m3
unscaled (N(0,1) fits e4m3's range).  Measured end-to-end rel-err 1.62e-2.

Schedule (cost-model verified, PE gap-free):
  - All activations/weights arrive partition-major so every DMA has >=1-2 KiB
    contiguous lines and one HWDGE slot per MiB-scale chunk.
  - DMA issue order feeds the first chains (fp8 weights + fp8 activations
    first - the opening gate is the pure-fp8 one) before everything else.
  - Gates run (o4, o1, o2, o3) steady-state; per weight tile the two 512-wide
    batch chunks are interleaved (one stationary load feeds two matmuls).  The
    final jt is bh-sequential ending on o4, split 2x256 so the last store is
    128 KiB.
"""

import numpy as np
import ml_dtypes

import concourse.bass as bass
import concourse.bacc as bacc
import concourse.mybir as mybir
from concourse.tile import TileContext
from concourse.bass_utils import run_bass_kernel_spmd

NCORES = 8
B, IN, OUT = 8192, 2048, 2048
K = IN + OUT                    # 4096 contraction dim
BLOC = B // NCORES              # 1024 batch rows per core
JT = OUT // 128                 # 16 output-dim tiles per gate
NBH = BLOC // 512               # 2 batch chunks of 512

# Per-gate fp8 contraction prefix (multiples of 256), tuned so
# max(rel_err_h, rel_err_c) = 1.62e-2 < 2e-2 budget.
K8S = (2560, 4096, 2048, 1536)
T8S = tuple(k // 256 for k in K8S)          # fp8 DoubleRow matmuls per chain
T8MAX = max(T8S)                            # 16 (gate 1 is pure fp8)
KBASE = min(K8S)                            # bf16 ih tiles cover [KBASE, K)
KBTS = tuple((K - k) // 128 for k in K8S)   # bf16 k-tiles per gate chain
KBOFF = tuple((k - KBASE) // 128 for k in K8S)  # first bf16 ih tile per gate
KBT_MAX = (K - KBASE) // 128                # 20
W8OFF = tuple(sum(t * 256 for t in T8S[:g]) for g in range(5))   # cols in w8 slab
WBOFF = tuple(sum(t * 128 for t in KBTS[:g]) for g in range(5))  # cols in wb slab
NKC = 10                        # bf16 k-tiles per ih DMA chunk
NCH = KBT_MAX // NKC            # 2 chunks per batch half

WSCALE = float(2.0 ** 12)       # host-side weight scale (fp8 normalization)
ASCALE = float(2.0 ** -12)      # undone in the gate activation

F32 = mybir.dt.float32
BF16 = mybir.dt.bfloat16
F8 = mybir.dt.float8e4
NPBF16 = ml_dtypes.bfloat16
NPF8 = ml_dtypes.float8_e4m3fn
DR = mybir.MatmulPerfMode.DoubleRow

# Steady-state gate order: o4 (new_h path) first so its epilogue overlaps
# later chains.  jt0/bh0 opens with the pure-fp8 gate (smallest DMA prefix).
GORDER = (3, 0, 1, 2)
GORDER_FIRST = (1, 3, 0, 2)
# Final batch chunk: end on o4 so only act -> mul -> store trail the last matmul.
GORDER_LAST = (0, 1, 2, 3)


def _build():
    nc = bacc.Bacc("TRN2", target_bir_lowering=False, debug=False, num_devices=NCORES)
    w8 = nc.declare_dram_parameter("w8", [JT, 128, W8OFF[4]], F8, isOutput=False)
    wb = nc.declare_dram_parameter("wb", [JT, 128, WBOFF[4]], BF16, isOutput=False)
    ih8d = [nc.declare_dram_parameter(f"ih8{bh}", [128, T8MAX, 2, 512], F8,
                                      isOutput=False) for bh in range(NBH)]
    ihbd = [nc.declare_dram_parameter(f"ihb{bh}", [128, KBT_MAX * 512], BF16,
                                      isOutput=False) for bh in range(NBH)]
    ct = nc.declare_dram_parameter("cT", [OUT, BLOC], F32, isOutput=False)
    bias = nc.declare_dram_parameter("bias", [128, 4 * JT], F32, isOutput=False)
    hT = nc.declare_dram_parameter("hT", [OUT, BLOC], F32, isOutput=True)
    cTo = nc.declare_dram_parameter("cTo", [OUT, BLOC], F32, isOutput=True)

    SIG = mybir.ActivationFunctionType.Sigmoid
    TANH = mybir.ActivationFunctionType.Tanh

    with TileContext(nc) as tc:
        with (
            tc.tile_pool(name="ihp", bufs=1) as ihp,
            tc.tile_pool(name="wp", bufs=2) as wp,
            tc.tile_pool(name="bp", bufs=1) as bp,
            tc.tile_pool(name="cp", bufs=2) as cp,
            tc.tile_pool(name="op", bufs=2) as op,
            tc.tile_pool(name="ep", bufs=3) as ep,
            tc.tile_pool(name="ps", bufs=8, space="PSUM") as psp,
        ):
            # --- jt=0 critical-path DMA ordering ---
            ih8t = [None, None]
            ihb_t = [[None, None] for _ in range(KBT_MAX)]

            def load_ih8(bh, t0, t1):
                if ih8t[bh] is None:
                    ih8t[bh] = ihp.tile([128, T8MAX, 2, 512], F8,
                                        tag=f"ih8b{bh}", name=f"ih8b{bh}")
                nc.sync.dma_start(out=ih8t[bh][:, t0:t1],
                                  in_=ih8d[bh][:, t0:t1])

            def load_ihb_chunk(bh, c):
                t = ihp.tile([128, NKC * 512], BF16,
                             tag=f"ihb{bh}c{c}", name=f"ihb{bh}c{c}")
                nc.sync.dma_start(
                    out=t, in_=ihbd[bh][:, c * NKC * 512:(c + 1) * NKC * 512])
                for i in range(NKC):
                    ihb_t[c * NKC + i][bh] = t[:, i * 512:(i + 1) * 512]

            wt = {}
            w8t = None

            def load_w8(jt):
                nonlocal w8t
                w8t = wp.tile([128, W8OFF[4]], F8, tag="w8", name="w8")
                nc.sync.dma_start(out=w8t, in_=w8[jt])

            def load_wb(jt, g):
                if KBTS[g] == 0:
                    return
                wt[g] = wp.tile([128, KBTS[g] * 128], BF16, tag=f"w{g}", name=f"w{g}")
                nc.sync.dma_start(
                    out=wt[g], in_=wb[jt][:, WBOFF[g]:WBOFF[g] + KBTS[g] * 128])

            load_w8(0)
            load_ih8(0, 0, T8MAX // 2)
            # bias is only needed by the first activation; keep its HWDGE slot
            # off the critical path of the first chain.
            bias_t = bp.tile([128, 4 * JT], F32)
            nc.sync.dma_start(out=bias_t, in_=bias[:, :])
            load_ih8(0, T8MAX // 2, T8MAX)
            load_wb(0, 3)
            load_ihb_chunk(0, 0)
            load_ihb_chunk(0, 1)
            load_wb(0, 0)
            load_wb(0, 2)
            ct0 = cp.tile([128, 512], F32, tag="c0")
            nc.sync.dma_start(out=ct0, in_=ct[0:128, 0:512])
            load_ih8(1, 0, T8MAX)
            for c in range(NCH):
                load_ihb_chunk(1, c)
            ct1 = cp.tile([128, 512], F32, tag="c1")
            nc.sync.dma_start(out=ct1, in_=ct[0:128, 512:1024])

            def epilogue_start(ctile):
                # tanh(c) on ScalarE, queued ahead of this chunk's gate
                # activations so it runs while PE is still in the chains.
                tanhc = op.tile([128, 512], F32, tag="tanhc")
                nc.scalar.activation(tanhc, ctile, TANH)
                return tanhc

            def gate_act(ps, jt, g, bh):
                o = op.tile([128, 512], F32, tag=f"o{g}b{bh}")
                nc.scalar.activation(
                    o, ps, TANH if g == 2 else SIG,
                    bias=bias_t[:, jt * 4 + g: jt * 4 + g + 1],
                    scale=ASCALE,
                )
                return o

            def epilogue_piece(st, g, o):
                # st: dict with ctile, tanhc, jsl, bsl; accumulates t1/o2.
                if g == 3:
                    nht = ep.tile([128, 512], F32, tag="nht")
                    nc.vector.tensor_mul(nht, st["tanhc"], o)
                    nc.sync.dma_start(out=hT[st["jsl"], st["bsl"]], in_=nht)
                elif g == 0:
                    t1 = ep.tile([128, 512], F32, tag="t1")
                    nc.vector.tensor_mul(t1, st["ctile"], o)
                    st["t1"] = t1
                elif g == 1:
                    st["o2"] = o
                elif g == 2:
                    t2 = ep.tile([128, 512], F32, tag="t2")
                    nc.vector.tensor_mul(t2, st["o2"], o)
                    nct = ep.tile([128, 512], F32, tag="nct")
                    nc.vector.tensor_add(nct, st["t1"], t2)
                    nc.sync.dma_start(out=cTo[st["jsl"], st["bsl"]], in_=nct)

            def chain_mms(pss, g, bhs, cols=None):
                # fp8 DoubleRow prefix then bf16 suffix, accumulating in one
                # PSUM bank per batch chunk; batch chunks interleaved so each
                # stationary load feeds len(bhs) matmuls.
                csl = slice(0, 512) if cols is None else cols
                n = csl.stop - csl.start
                for t in range(T8S[g]):
                    lhsT = w8t[:, W8OFF[g] + t * 256: W8OFF[g] + (t + 1) * 256]
                    lhsT = lhsT.rearrange("p (two f) -> p two f", two=2)
                    for ps, bh in zip(pss, bhs):
                        nc.tensor.matmul(
                            ps[:, 0:n],
                            lhsT=lhsT,
                            rhs=ih8t[bh][:, t, :, csl],
                            start=(t == 0),
                            stop=(t == T8S[g] - 1 and KBTS[g] == 0),
                            perf_mode=DR,
                        )
                for i in range(KBTS[g]):
                    kb = KBOFF[g] + i
                    for ps, bh in zip(pss, bhs):
                        nc.tensor.matmul(
                            ps[:, 0:n],
                            lhsT=wt[g][:, i * 128:(i + 1) * 128],
                            rhs=ihb_t[kb][bh][:, csl],
                            start=False,
                            stop=(i == KBTS[g] - 1),
                        )

            def bh_sequential(jt, ctiles, gorder):
                jsl = slice(jt * 128, (jt + 1) * 128)
                for bh in range(NBH):
                    st = {"ctile": ctiles[bh], "jsl": jsl,
                          "bsl": slice(bh * 512, (bh + 1) * 512)}
                    st["tanhc"] = epilogue_start(ctiles[bh])
                    for g in gorder:
                        ps = psp.tile([128, 512], F32, tag="ps")
                        chain_mms([ps], g, [bh])
                        epilogue_piece(st, g, gate_act(ps, jt, g, bh))

            # jt = 0: bh-sequential; opens with the pure-fp8 gate whose DMA
            # prefix (w8 + fp8 activations) is smallest.
            bh_sequential(0, (ct0, ct1), GORDER_FIRST)

            # 1 <= jt < JT-1: batch chunks interleaved per weight tile.
            for jt in range(1, JT - 1):
                jsl = slice(jt * 128, (jt + 1) * 128)
                load_w8(jt)
                for g in GORDER:
                    load_wb(jt, g)
                sts = []
                for bh in range(NBH):
                    ctile = cp.tile([128, 512], F32, tag=f"c{bh}")
                    nc.sync.dma_start(
                        out=ctile, in_=ct[jsl, bh * 512:(bh + 1) * 512])
                    st = {"ctile": ctile, "jsl": jsl,
                          "bsl": slice(bh * 512, (bh + 1) * 512)}
                    st["tanhc"] = epilogue_start(ctile)
                    sts.append(st)
                for g in GORDER:
                    pss = [psp.tile([128, 512], F32, tag="ps", name=f"ps{bh}")
                           for bh in range(NBH)]
                    chain_mms(pss, g, list(range(NBH)))
                    for bh in range(NBH):
                        epilogue_piece(sts[bh], g, gate_act(pss[bh], jt, g, bh))

            # Final jt: bh-sequential; bh=1 ends on o4 split into two 256-wide
            # halves so the first half's epilogue+store hides under the second
            # half's matmuls and the final store is only 128 KiB.
            jt = JT - 1
            jsl = slice(jt * 128, (jt + 1) * 128)
            load_w8(jt)
            for g in GORDER:
                load_wb(jt, g)
            ctiles = []
            for bh in range(NBH):
                ctile = cp.tile([128, 512], F32, tag=f"c{bh}")
                nc.sync.dma_start(out=ctile, in_=ct[jsl, bh * 512:(bh + 1) * 512])
                ctiles.append(ctile)

            st = {"ctile": ctiles[0], "jsl": jsl, "bsl": slice(0, 512)}
            st["tanhc"] = epilogue_start(ctiles[0])
            for g in GORDER:
                ps = psp.tile([128, 512], F32, tag="ps")
                chain_mms([ps], g, [0])
                epilogue_piece(st, g, gate_act(ps, jt, g, 0))

            st = {"ctile": ctiles[1], "jsl": jsl, "bsl": slice(512, 1024)}
            st["tanhc"] = epilogue_start(ctiles[1])
            for g in GORDER_LAST[:3]:
                ps = psp.tile([128, 512], F32, tag="ps")
                chain_mms([ps], g, [1])
                epilogue_piece(st, g, gate_act(ps, jt, g, 1))
            for half in range(2):
                cols = slice(half * 256, (half + 1) * 256)
                ps = psp.tile([128, 512], F32, tag="ps")
                chain_mms([ps], 3, [1], cols=cols)
                o = op.tile([128, 256], F32, tag=f"o3h{half}", name=f"o3h{half}")
                nc.scalar.activation(
                    o, ps[:, 0:256], SIG,
                    bias=bias_t[:, jt * 4 + 3: jt * 4 + 4],
                    scale=ASCALE,
                )
                nht = ep.tile([128, 256], F32, tag=f"nhth{half}", name=f"nhth{half}")
                nc.vector.tensor_mul(nht, st["tanhc"][:, cols], o)
                nc.sync.dma_start(
                    out=hT[jsl, 512 + half * 256: 512 + (half + 1) * 256],
                    in_=nht)
    nc.compile()
    return nc


def _prep_inputs(i, h, c, W1, b1, W2, b2, W3, b3, W4, b4):
    ih = np.concatenate([np.asarray(i, np.float32), np.asarray(h, np.float32)], axis=1)
    W4s = np.stack([np.asarray(W1), np.asarray(W2), np.asarray(W3), np.asarray(W4)])
    W4s = W4s.astype(np.float32) * WSCALE      # exact power-of-two scale

    # w8pack[jt, p, W8OFF[g] + t*256 + kt*128 + j] = e4m3(S*W_g[jt*128+j, (2t+kt)*128+p])
    w8parts = []
    wbparts = []
    for g in range(4):
        K8 = K8S[g]
        w8parts.append(np.ascontiguousarray(
            W4s[g, :, :K8].reshape(JT, 128, T8S[g], 2, 128)
            .transpose(0, 4, 2, 3, 1).reshape(JT, 128, T8S[g] * 256)))
        if KBTS[g]:
            # wbpack[jt, p, WBOFF[g] + i*128 + j] = bf16(S*W_g[jt*128+j, K8+i*128+p])
            wbparts.append(np.ascontiguousarray(
                W4s[g, :, K8:].reshape(JT, 128, KBTS[g], 128)
                .transpose(0, 3, 2, 1).reshape(JT, 128, KBTS[g] * 128)))
    w8pack = np.concatenate(w8parts, axis=2).astype(NPF8)
    wbpack = np.concatenate(wbparts, axis=2).astype(NPBF16)

    b4s = np.stack([np.asarray(b1), np.asarray(b2), np.asarray(b3), np.asarray(b4)])
    # biaspack[p, jt*4 + g] = b_g[jt*128 + p]   (unscaled: applied after scale)
    biaspack = np.ascontiguousarray(
        b4s.reshape(4, JT, 128).transpose(2, 1, 0).reshape(128, JT * 4)
    ).astype(np.float32)
    c = np.asarray(c, np.float32)

    in_maps = []
    for cs in range(NCORES):
        rows = slice(cs * BLOC, (cs + 1) * BLOC)
        ihT = np.ascontiguousarray(ih[rows].T)     # [K, BLOC] fp32
        cT = np.ascontiguousarray(c[rows].T)
        in_map = {"w8": w8pack, "wb": wbpack, "cT": cT, "bias": biaspack}
        for bh in range(NBH):
            half = ihT[:, bh * 512:(bh + 1) * 512]
            # ih8pack[p, t, kt, b] = e4m3(ihT[(2t+kt)*128+p, b])
            in_map[f"ih8{bh}"] = np.ascontiguousarray(
                half.reshape(T8MAX, 2, 128, 512).transpose(2, 0, 1, 3)
            ).astype(NPF8)
            # ihbpack[p, i*512 + b] = bf16(ihT[KBASE + i*128+p, b])
            in_map[f"ihb{bh}"] = np.ascontiguousarray(
                half[KBASE:].reshape(KBT_MAX, 128, 512).transpose(1, 0, 2)
            ).astype(NPBF16).reshape(128, KBT_MAX * 512)
        in_maps.append(in_map)
    return in_maps


def run_full(i, h, c, W1, b1, W2, b2, W3, b3, W4, b4, trace=False, **trace_kw):
    in_maps = _prep_inputs(i, h, c, W1, b1, W2, b2, W3, b3, W4, b4)
    nc = _build()
    r = run_bass_kernel_spmd(nc, in_maps, list(range(NCORES)), trace=trace, **trace_kw)
    hT = np.concatenate([r.results[cs]["hT"] for cs in range(NCORES)], axis=1)
    cTo = np.concatenate([r.results[cs]["cTo"] for cs in range(NCORES)], axis=1)
    new_h = np.ascontiguousarray(hT.T)
    new_c = np.ascontiguousarray(cTo.T)
    return (new_h, new_c), r


def kernel(i, h, c, W1, b1, W2, b2, W3, b3, W4, b4):
    out, _ = run_full(i, h, c, W1, b1, W2, b2, W3, b3, W4, b4, trace=False)
    return out


# revision 24
# speedup vs baseline: 1.1166x; 1.1166x over previous
"""LSTM cell (4-gate) Trainium2 Bass kernel, data-parallel over batch on 8 cores.

Computation (per reference):
    ih = concat(i, h, axis=1)                 # [B, K], K = 4096
    o_g = act_g(ih @ Wg.T + bg)               # gates, act = sigmoid/sigmoid/tanh/sigmoid
    new_c = c*o1 + o2*o3
    new_h = tanh(c) * o4

Strategy: shard batch B=8192 across 8 cores (1024 rows each); weights replicated.
All matmuls run in the transposed domain: out[j, b] = sum_k W_g[j, k] * ihT[k, b],
with the weight tile stationary and ihT moving [128k x 512b].  The gate bias is a
per-partition vector fused into the ScalarE activation.

Mixed precision, tuned PER GATE to the 2e-2 error budget: the first K8S[g]
columns of gate g's contraction run as fp8-e4m3 DoubleRow matmuls (256-wide
contraction per instruction at 2x PE rate); the rest run in bf16.  Gate
sensitivities differ a lot: o2's error is doubly damped (sigmoid' and |o3|<1)
so gate 1 is pure fp8; o1 is amplified by c (tail ~5.4) and o3 by tanh'=1, so
gates 0/2 keep larger bf16 suffixes; o4 alone sets the h error.  ALL weights
are pre-scaled by 2^12 on the host so fp8 weights sit in e4m3's normal range
while both partial products accumulate at the same scale in one PSUM bank; the
ScalarE activation applies scale=2^-12 for free.  ih is quantized to e4m3
unscaled (N(0,1) fits e4m3's range).  Measured end-to-end rel-err 1.62e-2.

Schedule (cost-model verified, PE gap-free):
  - All activations/weights arrive partition-major so every DMA has >=1-2 KiB
    contiguous lines and one HWDGE slot per MiB-scale chunk.
  - DMA issue order feeds the first chains (fp8 weights + fp8 activations
    first - the opening gate is the pure-fp8 one) before everything else.
  - Gates run (o4, o1, o2, o3) steady-state; per weight tile the two 512-wide
    batch chunks are interleaved (one stationary load feeds two matmuls).  The
    final jt is bh-sequential ending on o4, split 2x256 so the last store is
    128 KiB.
"""

import numpy as np
import ml_dtypes

import concourse.bass as bass
import concourse.bacc as bacc
import concourse.mybir as mybir
from concourse.tile import TileContext
from concourse.bass_utils import run_bass_kernel_spmd

NCORES = 8
B, IN, OUT = 8192, 2048, 2048
K = IN + OUT                    # 4096 contraction dim
BLOC = B // NCORES              # 1024 batch rows per core
JT = OUT // 128                 # 16 output-dim tiles per gate
NBH = BLOC // 512               # 2 batch chunks of 512

# Per-gate fp8 contraction prefix (multiples of 256), tuned so
# max(rel_err_h, rel_err_c) = 1.72e-2 < 2e-2 budget (13.9% margin, stable
# against reference precision and dataset re-rolls).
K8S = (3584, 4096, 2048, 1792)
T8S = tuple(k // 256 for k in K8S)          # fp8 DoubleRow matmuls per chain
T8MAX = max(T8S)                            # 16 (gate 1 is pure fp8)
KBASE = min(K8S)                            # bf16 ih tiles cover [KBASE, K)
KBTS = tuple((K - k) // 128 for k in K8S)   # bf16 k-tiles per gate chain
KBOFF = tuple((k - KBASE) // 128 for k in K8S)  # first bf16 ih tile per gate
KBT_MAX = (K - KBASE) // 128                # 20
W8OFF = tuple(sum(t * 256 for t in T8S[:g]) for g in range(5))   # cols in w8 slab
WBOFF = tuple(sum(t * 128 for t in KBTS[:g]) for g in range(5))  # cols in wb slab
NKC = 9                         # bf16 k-tiles per ih DMA chunk
NCH = KBT_MAX // NKC            # 2 chunks per batch half

WSCALE = float(2.0 ** 12)       # host-side weight scale (fp8 normalization)
ASCALE = float(2.0 ** -12)      # undone in the gate activation

F32 = mybir.dt.float32
BF16 = mybir.dt.bfloat16
F8 = mybir.dt.float8e4
NPBF16 = ml_dtypes.bfloat16
NPF8 = ml_dtypes.float8_e4m3fn
DR = mybir.MatmulPerfMode.DoubleRow

# Steady-state gate order: o4 (new_h path) first so its epilogue overlaps
# later chains.  jt0/bh0 opens with the pure-fp8 gate (smallest DMA prefix).
GORDER = (3, 0, 1, 2)
GORDER_FIRST = (1, 3, 0, 2)
# Final batch chunk: end on o4 so only act -> mul -> store trail the last matmul.
GORDER_LAST = (0, 1, 2, 3)


def _build():
    nc = bacc.Bacc("TRN2", target_bir_lowering=False, debug=False, num_devices=NCORES)
    w8 = nc.declare_dram_parameter("w8", [JT, 128, W8OFF[4]], F8, isOutput=False)
    wb = nc.declare_dram_parameter("wb", [JT, 128, WBOFF[4]], BF16, isOutput=False)
    ih8d = [nc.declare_dram_parameter(f"ih8{bh}", [128, T8MAX, 2, 512], F8,
                                      isOutput=False) for bh in range(NBH)]
    ihbd = [nc.declare_dram_parameter(f"ihb{bh}", [128, KBT_MAX * 512], BF16,
                                      isOutput=False) for bh in range(NBH)]
    ct = nc.declare_dram_parameter("cT", [OUT, BLOC], F32, isOutput=False)
    bias = nc.declare_dram_parameter("bias", [128, 4 * JT], F32, isOutput=False)
    hT = nc.declare_dram_parameter("hT", [OUT, BLOC], F32, isOutput=True)
    cTo = nc.declare_dram_parameter("cTo", [OUT, BLOC], F32, isOutput=True)

    SIG = mybir.ActivationFunctionType.Sigmoid
    TANH = mybir.ActivationFunctionType.Tanh

    with TileContext(nc) as tc:
        with (
            tc.tile_pool(name="ihp", bufs=1) as ihp,
            tc.tile_pool(name="wp", bufs=2) as wp,
            tc.tile_pool(name="bp", bufs=1) as bp,
            tc.tile_pool(name="cp", bufs=2) as cp,
            tc.tile_pool(name="op", bufs=2) as op,
            tc.tile_pool(name="ep", bufs=3) as ep,
            tc.tile_pool(name="ps", bufs=8, space="PSUM") as psp,
        ):
            # --- jt=0 critical-path DMA ordering ---
            ih8t = [None, None]
            ihb_t = [[None, None] for _ in range(KBT_MAX)]

            def load_ih8(bh, t0, t1):
                if ih8t[bh] is None:
                    ih8t[bh] = ihp.tile([128, T8MAX, 2, 512], F8,
                                        tag=f"ih8b{bh}", name=f"ih8b{bh}")
                nc.sync.dma_start(out=ih8t[bh][:, t0:t1],
                                  in_=ih8d[bh][:, t0:t1])

            def load_ihb_chunk(bh, c):
                t = ihp.tile([128, NKC * 512], BF16,
                             tag=f"ihb{bh}c{c}", name=f"ihb{bh}c{c}")
                nc.sync.dma_start(
                    out=t, in_=ihbd[bh][:, c * NKC * 512:(c + 1) * NKC * 512])
                for i in range(NKC):
                    ihb_t[c * NKC + i][bh] = t[:, i * 512:(i + 1) * 512]

            wt = {}
            w8t = None

            def load_w8(jt):
                nonlocal w8t
                w8t = wp.tile([128, W8OFF[4]], F8, tag="w8", name="w8")
                nc.sync.dma_start(out=w8t, in_=w8[jt])

            def load_wb(jt, g):
                if KBTS[g] == 0:
                    return
                wt[g] = wp.tile([128, KBTS[g] * 128], BF16, tag=f"w{g}", name=f"w{g}")
                nc.sync.dma_start(
                    out=wt[g], in_=wb[jt][:, WBOFF[g]:WBOFF[g] + KBTS[g] * 128])

            load_w8(0)
            load_ih8(0, 0, T8MAX // 2)
            # bias is only needed by the first activation; keep its HWDGE slot
            # off the critical path of the first chain.
            bias_t = bp.tile([128, 4 * JT], F32)
            nc.sync.dma_start(out=bias_t, in_=bias[:, :])
            load_ih8(0, T8MAX // 2, T8MAX)
            load_wb(0, 3)
            load_ihb_chunk(0, 0)
            load_ihb_chunk(0, 1)
            load_wb(0, 0)
            load_wb(0, 2)
            ct0 = cp.tile([128, 512], F32, tag="c0")
            nc.sync.dma_start(out=ct0, in_=ct[0:128, 0:512])
            load_ih8(1, 0, T8MAX)
            for c in range(NCH):
                load_ihb_chunk(1, c)
            ct1 = cp.tile([128, 512], F32, tag="c1")
            nc.sync.dma_start(out=ct1, in_=ct[0:128, 512:1024])

            def epilogue_start(ctile):
                # tanh(c) on ScalarE, queued ahead of this chunk's gate
                # activations so it runs while PE is still in the chains.
                tanhc = op.tile([128, 512], F32, tag="tanhc")
                nc.scalar.activation(tanhc, ctile, TANH)
                return tanhc

            def gate_act(ps, jt, g, bh):
                o = op.tile([128, 512], F32, tag=f"o{g}b{bh}")
                nc.scalar.activation(
                    o, ps, TANH if g == 2 else SIG,
                    bias=bias_t[:, jt * 4 + g: jt * 4 + g + 1],
                    scale=ASCALE,
                )
                return o

            def epilogue_piece(st, g, o):
                # st: dict with ctile, tanhc, jsl, bsl; accumulates t1/o2.
                if g == 3:
                    nht = ep.tile([128, 512], F32, tag="nht")
                    nc.vector.tensor_mul(nht, st["tanhc"], o)
                    nc.sync.dma_start(out=hT[st["jsl"], st["bsl"]], in_=nht)
                elif g == 0:
                    t1 = ep.tile([128, 512], F32, tag="t1")
                    nc.vector.tensor_mul(t1, st["ctile"], o)
                    st["t1"] = t1
                elif g == 1:
                    st["o2"] = o
                elif g == 2:
                    t2 = ep.tile([128, 512], F32, tag="t2")
                    nc.vector.tensor_mul(t2, st["o2"], o)
                    nct = ep.tile([128, 512], F32, tag="nct")
                    nc.vector.tensor_add(nct, st["t1"], t2)
                    nc.sync.dma_start(out=cTo[st["jsl"], st["bsl"]], in_=nct)

            def chain_mms(pss, g, bhs, cols=None):
                # fp8 DoubleRow prefix then bf16 suffix, accumulating in one
                # PSUM bank per batch chunk; batch chunks interleaved so each
                # stationary load feeds len(bhs) matmuls.
                csl = slice(0, 512) if cols is None else cols
                n = csl.stop - csl.start
                for t in range(T8S[g]):
                    lhsT = w8t[:, W8OFF[g] + t * 256: W8OFF[g] + (t + 1) * 256]
                    lhsT = lhsT.rearrange("p (two f) -> p two f", two=2)
                    for ps, bh in zip(pss, bhs):
                        nc.tensor.matmul(
                            ps[:, 0:n],
                            lhsT=lhsT,
                            rhs=ih8t[bh][:, t, :, csl],
                            start=(t == 0),
                            stop=(t == T8S[g] - 1 and KBTS[g] == 0),
                            perf_mode=DR,
                        )
                for i in range(KBTS[g]):
                    kb = KBOFF[g] + i
                    for ps, bh in zip(pss, bhs):
                        nc.tensor.matmul(
                            ps[:, 0:n],
                            lhsT=wt[g][:, i * 128:(i + 1) * 128],
                            rhs=ihb_t[kb][bh][:, csl],
                            start=False,
                            stop=(i == KBTS[g] - 1),
                        )

            def bh_sequential(jt, ctiles, gorder):
                jsl = slice(jt * 128, (jt + 1) * 128)
                for bh in range(NBH):
                    st = {"ctile": ctiles[bh], "jsl": jsl,
                          "bsl": slice(bh * 512, (bh + 1) * 512)}
                    st["tanhc"] = epilogue_start(ctiles[bh])
                    for g in gorder:
                        ps = psp.tile([128, 512], F32, tag="ps")
                        chain_mms([ps], g, [bh])
                        epilogue_piece(st, g, gate_act(ps, jt, g, bh))

            # jt = 0: bh-sequential; opens with the pure-fp8 gate whose DMA
            # prefix (w8 + fp8 activations) is smallest.
            bh_sequential(0, (ct0, ct1), GORDER_FIRST)

            # 1 <= jt < JT-1: batch chunks interleaved per weight tile.
            for jt in range(1, JT - 1):
                jsl = slice(jt * 128, (jt + 1) * 128)
                load_w8(jt)
                for g in GORDER:
                    load_wb(jt, g)
                sts = []
                for bh in range(NBH):
                    ctile = cp.tile([128, 512], F32, tag=f"c{bh}")
                    nc.sync.dma_start(
                        out=ctile, in_=ct[jsl, bh * 512:(bh + 1) * 512])
                    st = {"ctile": ctile, "jsl": jsl,
                          "bsl": slice(bh * 512, (bh + 1) * 512)}
                    st["tanhc"] = epilogue_start(ctile)
                    sts.append(st)
                for g in GORDER:
                    pss = [psp.tile([128, 512], F32, tag="ps", name=f"ps{bh}")
                           for bh in range(NBH)]
                    chain_mms(pss, g, list(range(NBH)))
                    for bh in range(NBH):
                        epilogue_piece(sts[bh], g, gate_act(pss[bh], jt, g, bh))

            # Final jt: bh-sequential; bh=1 ends on o4 split into two 256-wide
            # halves so the first half's epilogue+store hides under the second
            # half's matmuls and the final store is only 128 KiB.
            jt = JT - 1
            jsl = slice(jt * 128, (jt + 1) * 128)
            load_w8(jt)
            for g in GORDER:
                load_wb(jt, g)
            ctiles = []
            for bh in range(NBH):
                ctile = cp.tile([128, 512], F32, tag=f"c{bh}")
                nc.sync.dma_start(out=ctile, in_=ct[jsl, bh * 512:(bh + 1) * 512])
                ctiles.append(ctile)

            st = {"ctile": ctiles[0], "jsl": jsl, "bsl": slice(0, 512)}
            st["tanhc"] = epilogue_start(ctiles[0])
            for g in GORDER:
                ps = psp.tile([128, 512], F32, tag="ps")
                chain_mms([ps], g, [0])
                epilogue_piece(st, g, gate_act(ps, jt, g, 0))

            st = {"ctile": ctiles[1], "jsl": jsl, "bsl": slice(512, 1024)}
            st["tanhc"] = epilogue_start(ctiles[1])
            for g in GORDER_LAST[:3]:
                ps = psp.tile([128, 512], F32, tag="ps")
                chain_mms([ps], g, [1])
                epilogue_piece(st, g, gate_act(ps, jt, g, 1))
            for half in range(2):
                cols = slice(half * 256, (half + 1) * 256)
                ps = psp.tile([128, 512], F32, tag="ps")
                chain_mms([ps], 3, [1], cols=cols)
                o = op.tile([128, 256], F32, tag=f"o3h{half}", name=f"o3h{half}")
                nc.scalar.activation(
                    o, ps[:, 0:256], SIG,
                    bias=bias_t[:, jt * 4 + 3: jt * 4 + 4],
                    scale=ASCALE,
                )
                nht = ep.tile([128, 256], F32, tag=f"nhth{half}", name=f"nhth{half}")
                nc.vector.tensor_mul(nht, st["tanhc"][:, cols], o)
                nc.sync.dma_start(
                    out=hT[jsl, 512 + half * 256: 512 + (half + 1) * 256],
                    in_=nht)
    nc.compile()
    return nc


def _prep_inputs(i, h, c, W1, b1, W2, b2, W3, b3, W4, b4):
    ih = np.concatenate([np.asarray(i, np.float32), np.asarray(h, np.float32)], axis=1)
    W4s = np.stack([np.asarray(W1), np.asarray(W2), np.asarray(W3), np.asarray(W4)])
    W4s = W4s.astype(np.float32) * WSCALE      # exact power-of-two scale

    # w8pack[jt, p, W8OFF[g] + t*256 + kt*128 + j] = e4m3(S*W_g[jt*128+j, (2t+kt)*128+p])
    w8parts = []
    wbparts = []
    for g in range(4):
        K8 = K8S[g]
        w8parts.append(np.ascontiguousarray(
            W4s[g, :, :K8].reshape(JT, 128, T8S[g], 2, 128)
            .transpose(0, 4, 2, 3, 1).reshape(JT, 128, T8S[g] * 256)))
        if KBTS[g]:
            # wbpack[jt, p, WBOFF[g] + i*128 + j] = bf16(S*W_g[jt*128+j, K8+i*128+p])
            wbparts.append(np.ascontiguousarray(
                W4s[g, :, K8:].reshape(JT, 128, KBTS[g], 128)
                .transpose(0, 3, 2, 1).reshape(JT, 128, KBTS[g] * 128)))
    w8pack = np.concatenate(w8parts, axis=2).astype(NPF8)
    wbpack = np.concatenate(wbparts, axis=2).astype(NPBF16)

    b4s = np.stack([np.asarray(b1), np.asarray(b2), np.asarray(b3), np.asarray(b4)])
    # biaspack[p, jt*4 + g] = b_g[jt*128 + p]   (unscaled: applied after scale)
    biaspack = np.ascontiguousarray(
        b4s.reshape(4, JT, 128).transpose(2, 1, 0).reshape(128, JT * 4)
    ).astype(np.float32)
    c = np.asarray(c, np.float32)

    in_maps = []
    for cs in range(NCORES):
        rows = slice(cs * BLOC, (cs + 1) * BLOC)
        ihT = np.ascontiguousarray(ih[rows].T)     # [K, BLOC] fp32
        cT = np.ascontiguousarray(c[rows].T)
        in_map = {"w8": w8pack, "wb": wbpack, "cT": cT, "bias": biaspack}
        for bh in range(NBH):
            half = ihT[:, bh * 512:(bh + 1) * 512]
            # ih8pack[p, t, kt, b] = e4m3(ihT[(2t+kt)*128+p, b])
            in_map[f"ih8{bh}"] = np.ascontiguousarray(
                half.reshape(T8MAX, 2, 128, 512).transpose(2, 0, 1, 3)
            ).astype(NPF8)
            # ihbpack[p, i*512 + b] = bf16(ihT[KBASE + i*128+p, b])
            in_map[f"ihb{bh}"] = np.ascontiguousarray(
                half[KBASE:].reshape(KBT_MAX, 128, 512).transpose(1, 0, 2)
            ).astype(NPBF16).reshape(128, KBT_MAX * 512)
        in_maps.append(in_map)
    return in_maps


def run_full(i, h, c, W1, b1, W2, b2, W3, b3, W4, b4, trace=False, **trace_kw):
    in_maps = _prep_inputs(i, h, c, W1, b1, W2, b2, W3, b3, W4, b4)
    nc = _build()
    r = run_bass_kernel_spmd(nc, in_maps, list(range(NCORES)), trace=trace, **trace_kw)
    hT = np.concatenate([r.results[cs]["hT"] for cs in range(NCORES)], axis=1)
    cTo = np.concatenate([r.results[cs]["cTo"] for cs in range(NCORES)], axis=1)
    new_h = np.ascontiguousarray(hT.T)
    new_c = np.ascontiguousarray(cTo.T)
    return (new_h, new_c), r


def kernel(i, h, c, W1, b1, W2, b2, W3, b3, W4, b4):
    out, _ = run_full(i, h, c, W1, b1, W2, b2, W3, b3, W4, b4, trace=False)
    return out
